# revision 26
# baseline (speedup 1.0000x reference)
"""GCN 2-layer encoder on 8 Trainium2 NeuronCores (Bass/Tile).

Strategy (graph/data parallel, per sharding hint):
 - Nodes sharded by contiguous range across 8 cores (dst side).
 - h1 = x @ W1 (bf16) on each core's shard; the shard is split into 4
   quarters and each quarter is AllGathered separately, so aggregation for
   source-bucket q (the concat of every rank's q-th quarter, 25k rows,
   int16-indexable) can start as soon as its own AllGather lands --
   collectives overlap phase A / the previous bucket's gathers.
 - Aggregation is bucket-major: messages gathered via dma_gather
   round-robin across all 4 SWDGE queues (descriptor generation on all 8
   Q7 cores concurrently -- it is the kernel's bottleneck), scattered into
   per-tile PSUM via matmuls with host-precomputed norm-weighted bf16
   one-hots (built on DVE they stall on the DVE<->GpSimd shared SBUF
   port), accumulated across buckets in an SBUF fp32 accumulator.
 - Layer-1 epilogue fuses relu(+b1) on ScalarE, projects by W2 on PE, and
   writes zero-padded [dst, 128] bf16 rows; quarter-AllGathers of that
   table fire as epilogue quarters complete; layer 2 reuses the same
   one-hots as lhsT and adds b2.
"""
import numpy as np

NCORES = 8
P = 128
NQUART = 4

_CACHE = {}


# ---------------------------------------------------------------- preprocessing
def _prep(edge_index, n_nodes, n_cores=NCORES):
    # self-loop edges are NOT materialized: their diagonal contribution
    # dinv^2[n]*h[n] is added per tile via one matmul against diag tiles.
    # deg still counts the implicit self-loop (reference semantics).
    src = edge_index[0].astype(np.int64)
    dst = edge_index[1].astype(np.int64)
    deg = np.bincount(dst, minlength=n_nodes).astype(np.float32) + 1.0
    dinv = (1.0 / np.sqrt(deg)).astype(np.float32)
    norm = (dinv[src] * dinv[dst]).astype(np.float32)

    shard = n_nodes // n_cores
    assert shard * n_cores == n_nodes
    qsz = shard // NQUART
    assert qsz * NQUART == shard
    bktsz = qsz * n_cores  # rows per gathered quarter table
    assert bktsz <= 2 ** 15
    ntiles = (shard + P - 1) // P
    nbkt = NQUART

    core = dst // shard
    tile_id = (dst % shard) // P
    dstlocal = (dst % shard) % P
    src_r = src // shard
    src_m = src % shard
    bucket = src_m // qsz
    idx16 = (src_r * qsz + (src_m % qsz)).astype(np.int16)

    counts = np.zeros((n_cores, nbkt, ntiles), dtype=np.int64)
    np.add.at(counts, (core, bucket, tile_id), 1)
    K = (counts.max(axis=0) + P - 1) // P  # chunks per (bucket, tile)

    # bucket-major group order: bucket q's gathers only need quarter-AG q
    run_off = np.zeros((nbkt, ntiles), dtype=np.int64)
    off = 0
    groups = []  # (chunk_start, n_chunks, bucket, tile)
    for b in range(nbkt):
        for t in range(ntiles):
            k = int(K[b, t])
            run_off[b, t] = off * P
            if k == 0:
                continue
            groups.append((off, k, b, t))
            off += k
    nchunks = off
    npad = nchunks * P

    nvisit = (K > 0).sum(axis=0)           # visits per tile
    visit_ord = np.cumsum(K > 0, axis=0) - 1  # visit ordinal of (b, t)

    idx_pad = np.full((n_cores, npad), -1, dtype=np.int16)
    dl_pad = np.zeros((n_cores, npad), dtype=np.int32)
    norm_pad = np.zeros((n_cores, npad), dtype=np.float32)
    for (s, k, b, t) in groups:
        idx_pad[:, s * P] = 0  # keep >=1 valid idx even for empty groups
    gcnt = np.zeros((n_cores, len(groups)), dtype=np.int32)
    for gi_, (s, k, b, t) in enumerate(groups):
        gcnt[:, gi_] = np.maximum(counts[:, b, t], 1)

    order = np.lexsort((tile_id, bucket, core))
    bucket_s, tile_sorted = bucket[order], tile_id[order]
    idx_s, dl_s, norm_s = idx16[order], dstlocal[order], norm[order]
    core_s = core[order]

    for c in range(n_cores):
        m = core_s == c
        bs, ts = bucket_s[m], tile_sorted[m]
        key = bs * ntiles + ts
        sort_idx = np.argsort(key, kind="stable")
        kk = key[sort_idx]
        boundary = np.r_[True, kk[1:] != kk[:-1]] if len(kk) else np.zeros(0, bool)
        grp_start = np.flatnonzero(boundary)
        within = np.arange(len(kk)) - np.repeat(
            grp_start, np.diff(np.r_[grp_start, len(kk)])
        )
        ranks = np.empty_like(key)
        ranks[sort_idx] = within
        slot = run_off[bs, ts] + ranks
        idx_pad[c, slot] = idx_s[m]
        dl_pad[c, slot] = dl_s[m]
        norm_pad[c, slot] = norm_s[m]

    assert nvisit.min() >= 1, "tile with no incoming edges"
    return dict(
        shard=shard, qsz=qsz, bktsz=bktsz, ntiles=ntiles, nbkt=nbkt, K=K,
        groups=groups, idx_pad=idx_pad, dl_pad=dl_pad, norm_pad=norm_pad,
        nchunks=nchunks, gcnt=gcnt, nvisit=nvisit, visit_ord=visit_ord,
        dinv=dinv,
    )


def _build_diag(pp, c):
    """diag[p, t, d] = dinv^2 of node (c*shard + t*128 + p) if p == d else 0;
    the per-tile rhs/lhsT that adds the self-loop (diagonal) contribution."""
    import ml_dtypes
    shard, ntiles = pp["shard"], pp["ntiles"]
    d2 = (pp["dinv"][c * shard:(c + 1) * shard] ** 2).astype(np.float32)
    d2 = np.pad(d2, (0, ntiles * P - shard))
    diag = np.zeros((P, ntiles, P), dtype=ml_dtypes.bfloat16)
    ti = np.repeat(np.arange(ntiles), P)
    pi = np.tile(np.arange(P), ntiles)
    diag[pi, ti, pi] = d2  # node i = ti[i]*128 + pi[i]
    return np.ascontiguousarray(diag)


def _pack_idx(idx_pad_c):
    """[npad] int16 -> [128, npad//16] wrapped in 16 partitions, replicated x8
    so every SWDGE queue's Q7 core pair finds them in its partitions."""
    npad = idx_pad_c.shape[0]
    t = idx_pad_c.reshape(npad // 16, 16).T
    return np.ascontiguousarray(np.tile(t, (8, 1)))


def _build_onehots(pp, c):
    """bf16 one-hots: oh[p, chunk, d] = norm of edge (chunk*128+p) if its
    dstlocal == d else 0."""
    import ml_dtypes
    nchunks = pp["nchunks"]
    dl = pp["dl_pad"][c].reshape(nchunks, P)
    nm = pp["norm_pad"][c].reshape(nchunks, P)
    oh = np.zeros((nchunks, P, P), dtype=ml_dtypes.bfloat16)
    ci = np.repeat(np.arange(nchunks), P)
    pi = np.tile(np.arange(P), nchunks)
    oh[ci, pi, dl.ravel()] = nm.ravel().astype(ml_dtypes.bfloat16)
    return np.ascontiguousarray(oh.transpose(1, 0, 2))


# ---------------------------------------------------------------- device build
def _build(pp, fin, fh, fo, nq=4):
    import concourse.bass as bass
    import concourse.bacc as bacc
    import concourse.tile as tile
    import concourse.mybir as mybir
    from concourse.tile_rust import add_dep_helper

    f32 = mybir.dt.float32
    bf16 = mybir.dt.bfloat16
    shard, qsz, bktsz = pp["shard"], pp["qsz"], pp["bktsz"]
    ntiles, nbkt = pp["ntiles"], pp["nbkt"]
    nchunks, K, groups = pp["nchunks"], pp["K"], pp["groups"]
    nvisit, visit_ord = pp["nvisit"], pp["visit_ord"]
    kmax = int(K.max())
    kin = fin // P
    ngroups = len(groups)

    import os
    scratch = int(os.environ.get("DMA_SCRATCH", "32768"))
    nc = bacc.Bacc("TRN2", target_bir_lowering=False, debug=False,
                   num_devices=NCORES, num_swdge_queues=nq,
                   dynamic_dma_scratch_size=scratch)
    xT = nc.dram_tensor("xT", [fin, shard], bf16, kind="ExternalInput")
    W1 = nc.dram_tensor("W1b", [fin, fh], bf16, kind="ExternalInput")
    W2 = nc.dram_tensor("W2b", [fh, fo], bf16, kind="ExternalInput")
    b1c = nc.dram_tensor("b1c", [fh, 1], f32, kind="ExternalInput")
    b2r = nc.dram_tensor("b2r", [P, fo], f32, kind="ExternalInput")
    idx_d = nc.dram_tensor("idxt", [P, nchunks * 8], mybir.dt.int16,
                           kind="ExternalInput")
    oh_d = nc.dram_tensor("oht", [P, nchunks, P], bf16, kind="ExternalInput")
    gcnt_d = nc.dram_tensor("gcnt", [P, ngroups], mybir.dt.int32,
                            kind="ExternalInput")
    diag_d = nc.dram_tensor("diag", [P, ntiles, P], bf16,
                            kind="ExternalInput")
    outp = nc.dram_tensor("outp", [shard, fo], f32, kind="ExternalOutput")

    xT_v = xT.ap().rearrange("(a p) n -> p a n", p=P)
    W1_v = W1.ap().rearrange("(a p) c -> p a c", p=P)

    with tile.TileContext(nc) as tc:
        with (
            tc.tile_pool(name="const", bufs=1) as constp,
            tc.tile_pool(name="dram", bufs=1, space="DRAM") as dram,
        ):
            w1_sb = constp.tile([P, kin, fh], bf16)
            nc.sync.dma_start(out=w1_sb[:], in_=W1_v[:])
            w2_sb = constp.tile([P, fo], bf16)
            nc.sync.dma_start(out=w2_sb[:], in_=W2.ap()[:])
            b1_sb = constp.tile([P, 1], f32)
            nc.sync.dma_start(out=b1_sb[:], in_=b1c.ap()[:])
            b2_sb = constp.tile([P, fo], f32)
            nc.sync.dma_start(out=b2_sb[:], in_=b2r.ap()[:])
            idx_all = constp.tile([P, nchunks * 8], mybir.dt.int16)
            cnt_sb = constp.tile([P, ngroups], mybir.dt.int32)
            diag_sb = constp.tile([P, ntiles, P], bf16)
            cnt_dma = None
            cnt_regs = [nc.gpsimd.alloc_register("gcntr0"),
                        nc.gpsimd.alloc_register("gcntr1")]
            prev_gather = [None, None]  # last gather using each register

            def load_agg_consts():
                nonlocal cnt_dma
                nc.sync.dma_start(out=idx_all[:], in_=idx_d.ap()[:])
                cnt_dma = nc.sync.dma_start(out=cnt_sb[:], in_=gcnt_d.ap()[:])
                nc.sync.dma_start(out=diag_sb[:], in_=diag_d.ap()[:])

            # per-quarter shard slabs and gathered tables
            h1q = [dram.tile([qsz, fh], bf16, name=f"h1q{q}")
                   for q in range(NQUART)]
            giq = [dram.tile([qsz, P], bf16, name=f"giq{q}")
                   for q in range(NQUART)]
            cur = {}

            def store_by_quarter(slabs, sb_tile, lo, nw):
                # route rows [lo, lo+nw) of the shard into quarter slabs
                r0 = lo
                while r0 < lo + nw:
                    q = r0 // qsz
                    r1 = min((q + 1) * qsz, lo + nw)
                    nc.sync.dma_start(
                        out=slabs[q][r0 - q * qsz:r1 - q * qsz, :],
                        in_=sb_tile[r0 - lo:r1 - lo, :])
                    r0 = r1

            def phase_a():
                blk = qsz + P  # quarter block padded to cover straddling tiles
                with (
                    tc.tile_pool(name="pa_x", bufs=2) as pa_x,
                    tc.tile_pool(name="pa_sb", bufs=4) as pa_sb,
                    tc.tile_pool(name="pa_ps", bufs=2, space="PSUM") as pa_ps,
                ):
                    xq, cur_q = None, -1
                    for t in range(ntiles):
                        lo = t * P
                        nw = min(P, shard - lo)
                        qt = lo // qsz
                        if qt != cur_q:
                            cur_q = qt
                            q0 = qt * qsz
                            qw = min(blk, shard - q0)
                            xq = pa_x.tile([P, kin, blk], bf16, tag="xq")
                            nc.sync.dma_start(out=xq[:, :, :qw],
                                              in_=xT_v[:, :, q0:q0 + qw])
                        off = lo - cur_q * qsz
                        ps = pa_ps.tile([P, fh], f32, tag="ps")
                        for a in range(kin):
                            nc.tensor.matmul(out=ps[:nw, :],
                                             lhsT=xq[:, a, off:off + nw],
                                             rhs=w1_sb[:, a, :], start=(a == 0),
                                             stop=(a == kin - 1))
                        hsb = pa_sb.tile([P, fh], bf16, tag="hsb")
                        nc.vector.tensor_copy(out=hsb[:nw, :], in_=ps[:nw, :])
                        store_by_quarter(h1q, hsb, lo, nw)

            _agn = [0]

            def ag(src_t, width, key):
                dst_t = dram.tile([bktsz, width], bf16,
                                  name=f"{key}_{_agn[0]}", addr_space="Shared")
                _agn[0] += 1
                nc.gpsimd.collective_compute(
                    "AllGather", mybir.AluOpType.bypass,
                    replica_groups=[list(range(NCORES))],
                    ins=[src_t.opt()], outs=[dst_t.opt()],
                )
                cur[key] = dst_t

            def agg_pass(tables, local_slabs, layer1, acc_pool, epilogue):
                accs = acc_pool.tile([P, ntiles * P], f32,
                                     name=f"acc{int(layer1)}")
                with (
                    tc.tile_pool(name=f"gb{int(layer1)}", bufs=1) as gpool,
                    tc.tile_pool(name=f"oh{int(layer1)}", bufs=10) as ohpool,
                    tc.tile_pool(name=f"lt{int(layer1)}", bufs=3) as ltpool,
                    tc.tile_pool(name=f"ps{int(layer1)}", bufs=3,
                                 space="PSUM") as pspool,
                ):
                    gbufs = []
                    for bi_ in range(16):
                        gz = gpool.tile([P, kmax, fh], bf16, tag=f"gb{bi_}",
                                        name=f"gbuf{int(layer1)}_{bi_}")
                        nc.vector.memset(gz[:], 0.0)
                        gbufs.append(gz)
                    gctr = [0]
                    for gi_, (s, k, b, t) in enumerate(groups):
                        gb = gbufs[gctr[0] % 16]
                        r = gctr[0] % 2
                        ld = nc.gpsimd.load(cnt_regs[r],
                                            cnt_sb[0:1, gi_:gi_ + 1])
                        if prev_gather[r] is not None:
                            add_dep_helper(ld.ins, prev_gather[r].ins,
                                           sync=False, reason="gcnt reg WAR")
                        else:
                            add_dep_helper(ld.ins, cnt_dma.ins, sync=True,
                                           reason="gcnt sbuf RAW")
                        gth = nc.gpsimd.dma_gather(
                            out_ap=gb[:, :k, :],
                            in_ap=tables[b].opt()[0:bktsz, :],
                            idxs_ap=idx_all[:, s * 8:(s + k) * 8],
                            num_idxs=k * P,
                            num_idxs_reg=cnt_regs[r],
                            elem_size=fh,
                            single_packet=False,
                            queue_num=gctr[0] % 4,
                        )
                        add_dep_helper(gth.ins, ld.ins, sync=False,
                                       reason="gcnt reg RAW")
                        prev_gather[r] = gth
                        gctr[0] += 1
                        oh = ohpool.tile([P, kmax, P], bf16, tag="oh")
                        nc.sync.dma_start(out=oh[:, :k, :],
                                          in_=oh_d.ap()[:, s:s + k, :])
                        ps = pspool.tile([P, P], f32, tag="ps")
                        first_visit = visit_ord[b, t] == 0
                        if first_visit:
                            # self-loop diagonal: one matmul against the
                            # core-local table rows of this tile
                            lo = t * P
                            nw = min(P, shard - lo)
                            lt = ltpool.tile([P, fh], bf16, tag="lt")
                            r0 = lo
                            while r0 < lo + nw:
                                q = r0 // qsz
                                r1 = min((q + 1) * qsz, lo + nw)
                                nc.sync.dma_start(
                                    out=lt[r0 - lo:r1 - lo, :],
                                    in_=local_slabs[q][r0 - q * qsz:
                                                       r1 - q * qsz, :])
                                r0 = r1
                            if layer1:
                                nc.tensor.matmul(
                                    out=ps[:], lhsT=lt[:],
                                    rhs=diag_sb[:, t, :], start=True,
                                    stop=False)
                            else:
                                nc.tensor.matmul(
                                    out=ps[:], lhsT=diag_sb[:, t, :],
                                    rhs=lt[:], start=True, stop=False)
                        for j in range(k):
                            st = (j == 0) and not first_visit
                            if layer1:
                                nc.tensor.matmul(
                                    out=ps[:], lhsT=gb[:, j, :],
                                    rhs=oh[:, j, :], start=st,
                                    stop=(j == k - 1))
                            else:
                                nc.tensor.matmul(
                                    out=ps[:], lhsT=oh[:, j, :],
                                    rhs=gb[:, j, :], start=st,
                                    stop=(j == k - 1))
                        asl = accs[:, t * P:(t + 1) * P]
                        if visit_ord[b, t] == 0:
                            nc.vector.tensor_copy(out=asl, in_=ps[:])
                        else:
                            nc.vector.tensor_tensor(
                                out=asl, in0=asl, in1=ps[:],
                                op=mybir.AluOpType.add)
                        if visit_ord[b, t] == nvisit[t] - 1:
                            epilogue(t, asl)

            def phase_b():
                with (
                    tc.tile_pool(name="acc1p", bufs=1) as acc1p,
                    tc.tile_pool(name="ep1", bufs=1) as ep1,
                    tc.tile_pool(name="ep1ps", bufs=2, space="PSUM") as ep1ps,
                ):
                    gsb = []
                    for i in range(3):
                        g = ep1.tile([P, P], bf16, tag=f"gsb{i}",
                                     name=f"gsb{i}")
                        nc.vector.memset(g[:], 0.0)
                        gsb.append(g)
                    zpool = [ep1.tile([P, P], bf16, tag=f"z{i}", name=f"zb{i}")
                             for i in range(3)]

                    def epi1(t, asl):
                        lo = t * P
                        nw = min(P, shard - lo)
                        z = zpool[t % 3]
                        nc.scalar.activation(
                            out=z[:], in_=asl,
                            func=mybir.ActivationFunctionType.Relu,
                            bias=b1_sb[:, 0:1])
                        ps2 = ep1ps.tile([P, fo], f32, tag="ps2")
                        nc.tensor.matmul(out=ps2[:], lhsT=z[:], rhs=w2_sb[:],
                                         start=True, stop=True)
                        g = gsb[t % 3]
                        nc.vector.tensor_copy(out=g[:, :fo], in_=ps2[:])
                        store_by_quarter(giq, g, lo, nw)

                    agg_pass([cur[f"h1f{q}"] for q in range(NQUART)], h1q,
                             True, acc1p, epi1)

            def phase_d():
                with (
                    tc.tile_pool(name="acc2p", bufs=1) as acc2p,
                    tc.tile_pool(name="ep2", bufs=3) as ep2,
                ):
                    def epi2(t, asl):
                        lo = t * P
                        nw = min(P, shard - lo)
                        o = ep2.tile([P, fo], f32, tag="o")
                        nc.vector.tensor_tensor(out=o[:], in0=asl[:, :fo],
                                                in1=b2_sb[:],
                                                op=mybir.AluOpType.add)
                        nc.sync.dma_start(out=outp.ap()[lo:lo + nw, :],
                                          in_=o[:nw, :])

                    agg_pass([cur[f"gf{q}"] for q in range(NQUART)], giq,
                             False, acc2p, epi2)

            phase_a()
            load_agg_consts()
            for q in range(NQUART):
                ag(h1q[q], fh, f"h1f{q}")
            phase_b()
            for q in range(NQUART):
                ag(giq[q], P, f"gf{q}")
            phase_d()

    nc.compile()
    return nc


# ---------------------------------------------------------------- entry point
def kernel(x, edge_index, W1, b1, W2, b2, _want_results=False, _trace=False):
    import ml_dtypes
    import concourse.bass_utils as bass_utils

    x = np.ascontiguousarray(np.asarray(x, dtype=np.float32))
    ei = np.asarray(edge_index).astype(np.int64)
    W1 = np.asarray(W1, dtype=np.float32)
    b1 = np.asarray(b1, dtype=np.float32)
    W2 = np.asarray(W2, dtype=np.float32)
    b2 = np.asarray(b2, dtype=np.float32)
    n, fin = x.shape
    fh = W1.shape[1]
    fo = W2.shape[1]

    key = ("v7", n, fin, fh, fo, int(ei[0, :8].sum()), int(ei[1, :8].sum()),
           ei.shape[1])
    if key in _CACHE:
        nc, pp, in_static = _CACHE[key]
    else:
        pp = _prep(ei, n)
        nc = _build(pp, fin, fh, fo)
        in_static = []
        for c in range(NCORES):
            in_static.append({
                "idxt": _pack_idx(pp["idx_pad"][c]),
                "oht": _build_onehots(pp, c),
                "gcnt": np.ascontiguousarray(
                    np.tile(pp["gcnt"][c:c + 1], (P, 1))),
                "diag": _build_diag(pp, c),
            })
        _CACHE[key] = (nc, pp, in_static)

    shard = pp["shard"]
    b1c = b1.reshape(fh, 1)
    b2r = np.tile(b2[None, :], (P, 1))
    W1b = W1.astype(ml_dtypes.bfloat16)
    W2b = W2.astype(ml_dtypes.bfloat16)

    in_maps = []
    for c in range(NCORES):
        xT = np.ascontiguousarray(
            x[c * shard:(c + 1) * shard, :].T.astype(ml_dtypes.bfloat16))
        m = {"xT": xT, "W1b": W1b, "W2b": W2b, "b1c": b1c, "b2r": b2r}
        m.update(in_static[c])
        in_maps.append(m)

    res = bass_utils.run_bass_kernel_spmd(
        nc, in_maps, core_ids=list(range(NCORES)), trace=_trace)
    out = np.concatenate([res.results[c]["outp"] for c in range(NCORES)],
                         axis=0)
    if _want_results:
        return out, res
    return out


# revision 29
# speedup vs baseline: 1.0227x; 1.0227x over previous
"""GCN 2-layer encoder on 8 Trainium2 NeuronCores (Bass/Tile).

Strategy (graph/data parallel, per sharding hint):
 - Nodes sharded by contiguous range across 8 cores (dst side).
 - h1 = x @ W1 (bf16) on each core's shard; the shard is split into 4
   quarters and each quarter is AllGathered separately, so aggregation for
   source-bucket q (the concat of every rank's q-th quarter, 25k rows,
   int16-indexable) can start as soon as its own AllGather lands --
   collectives overlap phase A / the previous bucket's gathers.
 - Aggregation is bucket-major: messages gathered via dma_gather
   round-robin across all 4 SWDGE queues (descriptor generation on all 8
   Q7 cores concurrently -- it is the kernel's bottleneck), scattered into
   per-tile PSUM via matmuls with host-precomputed norm-weighted bf16
   one-hots (built on DVE they stall on the DVE<->GpSimd shared SBUF
   port), accumulated across buckets in an SBUF fp32 accumulator.
 - Layer-1 epilogue fuses relu(+b1) on ScalarE, projects by W2 on PE, and
   writes zero-padded [dst, 128] bf16 rows; quarter-AllGathers of that
   table fire as epilogue quarters complete; layer 2 reuses the same
   one-hots as lhsT and adds b2.
"""
import numpy as np

NCORES = 8
P = 128
NQUART = 4

_CACHE = {}


# ---------------------------------------------------------------- preprocessing
def _prep(edge_index, n_nodes, n_cores=NCORES):
    # self-loop edges are NOT materialized: their diagonal contribution
    # dinv^2[n]*h[n] is added per tile via one matmul against diag tiles.
    # deg still counts the implicit self-loop (reference semantics).
    src = edge_index[0].astype(np.int64)
    dst = edge_index[1].astype(np.int64)
    deg = np.bincount(dst, minlength=n_nodes).astype(np.float32) + 1.0
    dinv = (1.0 / np.sqrt(deg)).astype(np.float32)
    norm = (dinv[src] * dinv[dst]).astype(np.float32)

    shard = n_nodes // n_cores
    assert shard * n_cores == n_nodes
    qsz = shard // NQUART
    assert qsz * NQUART == shard
    bktsz = qsz * n_cores  # rows per gathered quarter table
    assert bktsz <= 2 ** 15
    ntiles = (shard + P - 1) // P
    nbkt = NQUART

    core = dst // shard
    tile_id = (dst % shard) // P
    dstlocal = (dst % shard) % P
    src_r = src // shard
    src_m = src % shard
    bucket = src_m // qsz
    idx16 = (src_r * qsz + (src_m % qsz)).astype(np.int16)

    counts = np.zeros((n_cores, nbkt, ntiles), dtype=np.int64)
    np.add.at(counts, (core, bucket, tile_id), 1)
    K = (counts.max(axis=0) + P - 1) // P  # chunks per (bucket, tile)

    # bucket-major group order: bucket q's gathers only need quarter-AG q
    run_off = np.zeros((nbkt, ntiles), dtype=np.int64)
    off = 0
    groups = []  # (chunk_start, n_chunks, bucket, tile)
    for b in range(nbkt):
        for t in range(ntiles):
            k = int(K[b, t])
            run_off[b, t] = off * P
            if k == 0:
                continue
            groups.append((off, k, b, t))
            off += k
    nchunks = off
    npad = nchunks * P

    nvisit = (K > 0).sum(axis=0)           # visits per tile
    visit_ord = np.cumsum(K > 0, axis=0) - 1  # visit ordinal of (b, t)

    idx_pad = np.full((n_cores, npad), -1, dtype=np.int16)
    dl_pad = np.zeros((n_cores, npad), dtype=np.int32)
    norm_pad = np.zeros((n_cores, npad), dtype=np.float32)
    for (s, k, b, t) in groups:
        idx_pad[:, s * P] = 0  # keep >=1 valid idx even for empty groups
    gcnt = np.zeros((n_cores, len(groups)), dtype=np.int32)
    gmax = np.zeros(len(groups), dtype=np.int64)
    for gi_, (s, k, b, t) in enumerate(groups):
        gcnt[:, gi_] = np.maximum(counts[:, b, t], 1)
        gmax[gi_] = counts[:, b, t].max()

    order = np.lexsort((tile_id, bucket, core))
    bucket_s, tile_sorted = bucket[order], tile_id[order]
    idx_s, dl_s, norm_s = idx16[order], dstlocal[order], norm[order]
    core_s = core[order]

    for c in range(n_cores):
        m = core_s == c
        bs, ts = bucket_s[m], tile_sorted[m]
        key = bs * ntiles + ts
        sort_idx = np.argsort(key, kind="stable")
        kk = key[sort_idx]
        boundary = np.r_[True, kk[1:] != kk[:-1]] if len(kk) else np.zeros(0, bool)
        grp_start = np.flatnonzero(boundary)
        within = np.arange(len(kk)) - np.repeat(
            grp_start, np.diff(np.r_[grp_start, len(kk)])
        )
        ranks = np.empty_like(key)
        ranks[sort_idx] = within
        slot = run_off[bs, ts] + ranks
        idx_pad[c, slot] = idx_s[m]
        dl_pad[c, slot] = dl_s[m]
        norm_pad[c, slot] = norm_s[m]

    assert nvisit.min() >= 1, "tile with no incoming edges"
    return dict(
        shard=shard, qsz=qsz, bktsz=bktsz, ntiles=ntiles, nbkt=nbkt, K=K,
        groups=groups, idx_pad=idx_pad, dl_pad=dl_pad, norm_pad=norm_pad,
        nchunks=nchunks, gcnt=gcnt, nvisit=nvisit, visit_ord=visit_ord,
        dinv=dinv, gmax=gmax,
    )


def _build_diag(pp, c):
    """diag[p, t, d] = dinv^2 of node (c*shard + t*128 + p) if p == d else 0;
    the per-tile rhs/lhsT that adds the self-loop (diagonal) contribution."""
    import ml_dtypes
    shard, ntiles = pp["shard"], pp["ntiles"]
    d2 = (pp["dinv"][c * shard:(c + 1) * shard] ** 2).astype(np.float32)
    d2 = np.pad(d2, (0, ntiles * P - shard))
    diag = np.zeros((P, ntiles, P), dtype=ml_dtypes.bfloat16)
    ti = np.repeat(np.arange(ntiles), P)
    pi = np.tile(np.arange(P), ntiles)
    diag[pi, ti, pi] = d2  # node i = ti[i]*128 + pi[i]
    return np.ascontiguousarray(diag)


def _pack_idx(idx_pad_c):
    """[npad] int16 -> [128, npad//16] wrapped in 16 partitions, replicated x8
    so every SWDGE queue's Q7 core pair finds them in its partitions."""
    npad = idx_pad_c.shape[0]
    t = idx_pad_c.reshape(npad // 16, 16).T
    return np.ascontiguousarray(np.tile(t, (8, 1)))


def _build_onehots(pp, c):
    """bf16 one-hots: oh[p, chunk, d] = norm of edge (chunk*128+p) if its
    dstlocal == d else 0."""
    import ml_dtypes
    nchunks = pp["nchunks"]
    dl = pp["dl_pad"][c].reshape(nchunks, P)
    nm = pp["norm_pad"][c].reshape(nchunks, P)
    oh = np.zeros((nchunks, P, P), dtype=ml_dtypes.bfloat16)
    ci = np.repeat(np.arange(nchunks), P)
    pi = np.tile(np.arange(P), nchunks)
    oh[ci, pi, dl.ravel()] = nm.ravel().astype(ml_dtypes.bfloat16)
    return np.ascontiguousarray(oh.transpose(1, 0, 2))


# ---------------------------------------------------------------- device build
def _build(pp, fin, fh, fo, nq=4):
    import concourse.bass as bass
    import concourse.bacc as bacc
    import concourse.tile as tile
    import concourse.mybir as mybir
    from concourse.tile_rust import add_dep_helper

    f32 = mybir.dt.float32
    bf16 = mybir.dt.bfloat16
    shard, qsz, bktsz = pp["shard"], pp["qsz"], pp["bktsz"]
    ntiles, nbkt = pp["ntiles"], pp["nbkt"]
    nchunks, K, groups = pp["nchunks"], pp["K"], pp["groups"]
    nvisit, visit_ord = pp["nvisit"], pp["visit_ord"]
    kmax = int(K.max())
    kin = fin // P
    ngroups = len(groups)

    import os
    scratch = int(os.environ.get("DMA_SCRATCH", "32768"))
    nc = bacc.Bacc("TRN2", target_bir_lowering=False, debug=False,
                   num_devices=NCORES, num_swdge_queues=nq,
                   dynamic_dma_scratch_size=scratch)
    xT = nc.dram_tensor("xT", [fin, shard], bf16, kind="ExternalInput")
    W1 = nc.dram_tensor("W1b", [fin, fh], bf16, kind="ExternalInput")
    W2 = nc.dram_tensor("W2b", [fh, fo], bf16, kind="ExternalInput")
    b1c = nc.dram_tensor("b1c", [fh, 1], f32, kind="ExternalInput")
    b2r = nc.dram_tensor("b2r", [P, fo], f32, kind="ExternalInput")
    idx_d = nc.dram_tensor("idxt", [P, nchunks * 8], mybir.dt.int16,
                           kind="ExternalInput")
    oh_d = nc.dram_tensor("oht", [P, nchunks, P], bf16, kind="ExternalInput")
    gcnt_d = nc.dram_tensor("gcnt", [P, ngroups], mybir.dt.int32,
                            kind="ExternalInput")
    diag_d = nc.dram_tensor("diag", [P, ntiles, P], bf16,
                            kind="ExternalInput")
    outp = nc.dram_tensor("outp", [shard, fo], f32, kind="ExternalOutput")

    xT_v = xT.ap().rearrange("(a p) n -> p a n", p=P)
    W1_v = W1.ap().rearrange("(a p) c -> p a c", p=P)

    with tile.TileContext(nc) as tc:
        with (
            tc.tile_pool(name="const", bufs=1) as constp,
            tc.tile_pool(name="dram", bufs=1, space="DRAM") as dram,
        ):
            w1_sb = constp.tile([P, kin, fh], bf16)
            nc.sync.dma_start(out=w1_sb[:], in_=W1_v[:])
            w2_sb = constp.tile([P, fo], bf16)
            nc.sync.dma_start(out=w2_sb[:], in_=W2.ap()[:])
            b1_sb = constp.tile([P, 1], f32)
            nc.sync.dma_start(out=b1_sb[:], in_=b1c.ap()[:])
            b2_sb = constp.tile([P, fo], f32)
            nc.sync.dma_start(out=b2_sb[:], in_=b2r.ap()[:])
            idx_all = constp.tile([P, nchunks * 8], mybir.dt.int16)
            cnt_sb = constp.tile([P, ngroups], mybir.dt.int32)
            diag_sb = constp.tile([P, ntiles, P], bf16)
            cnt_dma = None
            cnt_reg = nc.gpsimd.alloc_register("gcntr")
            prev_gather = [None]

            def load_agg_consts():
                nonlocal cnt_dma
                nc.sync.dma_start(out=idx_all[:], in_=idx_d.ap()[:])
                cnt_dma = nc.sync.dma_start(out=cnt_sb[:], in_=gcnt_d.ap()[:])
                nc.sync.dma_start(out=diag_sb[:], in_=diag_d.ap()[:])

            # per-quarter shard slabs and gathered tables
            h1q = [dram.tile([qsz, fh], bf16, name=f"h1q{q}")
                   for q in range(NQUART)]
            giq = [dram.tile([qsz, P], bf16, name=f"giq{q}")
                   for q in range(NQUART)]
            cur = {}

            def store_by_quarter(slabs, sb_tile, lo, nw):
                # route rows [lo, lo+nw) of the shard into quarter slabs
                r0 = lo
                while r0 < lo + nw:
                    q = r0 // qsz
                    r1 = min((q + 1) * qsz, lo + nw)
                    nc.sync.dma_start(
                        out=slabs[q][r0 - q * qsz:r1 - q * qsz, :],
                        in_=sb_tile[r0 - lo:r1 - lo, :])
                    r0 = r1

            def phase_a():
                blk = qsz + P  # quarter block padded to cover straddling tiles
                with (
                    tc.tile_pool(name="pa_x", bufs=2) as pa_x,
                    tc.tile_pool(name="pa_sb", bufs=8) as pa_sb,
                    tc.tile_pool(name="pa_ps", bufs=4, space="PSUM") as pa_ps,
                ):
                    xq, cur_q = None, -1
                    for t in range(ntiles):
                        lo = t * P
                        nw = min(P, shard - lo)
                        qt = lo // qsz
                        if qt != cur_q:
                            cur_q = qt
                            q0 = qt * qsz
                            qw = min(blk, shard - q0)
                            xq = pa_x.tile([P, kin, blk], bf16, tag="xq")
                            nc.sync.dma_start(out=xq[:, :, :qw],
                                              in_=xT_v[:, :, q0:q0 + qw])
                        off = lo - cur_q * qsz
                        ps = pa_ps.tile([P, fh], f32, tag="ps")
                        for a in range(kin):
                            nc.tensor.matmul(out=ps[:nw, :],
                                             lhsT=xq[:, a, off:off + nw],
                                             rhs=w1_sb[:, a, :], start=(a == 0),
                                             stop=(a == kin - 1))
                        hsb = pa_sb.tile([P, fh], bf16, tag="hsb")
                        nc.vector.tensor_copy(out=hsb[:nw, :], in_=ps[:nw, :])
                        store_by_quarter(h1q, hsb, lo, nw)

            _agn = [0]

            def ag(src_t, width, key):
                dst_t = dram.tile([bktsz, width], bf16,
                                  name=f"{key}_{_agn[0]}", addr_space="Shared")
                _agn[0] += 1
                nc.gpsimd.collective_compute(
                    "AllGather", mybir.AluOpType.bypass,
                    replica_groups=[list(range(NCORES))],
                    ins=[src_t.opt()], outs=[dst_t.opt()],
                )
                cur[key] = dst_t

            def agg_pass(tables, local_slabs, layer1, acc_pool, epilogue):
                accs = acc_pool.tile([P, ntiles * P], f32,
                                     name=f"acc{int(layer1)}")
                with (
                    tc.tile_pool(name=f"gb{int(layer1)}", bufs=1) as gpool,
                    tc.tile_pool(name=f"oh{int(layer1)}", bufs=6) as ohpool,
                    tc.tile_pool(name=f"lt{int(layer1)}", bufs=3) as ltpool,
                    tc.tile_pool(name=f"ps{int(layer1)}", bufs=3,
                                 space="PSUM") as pspool,
                ):
                    gbufs = []
                    for bi_ in range(16):
                        gz = gpool.tile([P, kmax, fh], bf16, tag=f"gb{bi_}",
                                        name=f"gbuf{int(layer1)}_{bi_}")
                        nc.vector.memset(gz[:], 0.0)
                        gbufs.append(gz)
                    gctr = [0]
                    for gi_, (s, k, b, t) in enumerate(groups):
                        gb = gbufs[gctr[0] % 16]
                        qq = gctr[0] % 4  # keep queue<->sem-lane congruence
                        ld = nc.gpsimd.load(cnt_reg, cnt_sb[0:1, gi_:gi_ + 1])
                        if prev_gather[0] is not None:
                            add_dep_helper(ld.ins, prev_gather[0].ins,
                                           sync=False, reason="gcnt reg WAR")
                        else:
                            add_dep_helper(ld.ins, cnt_dma.ins, sync=True,
                                           reason="gcnt sbuf RAW")
                        gth = nc.gpsimd.dma_gather(
                            out_ap=gb[:, :k, :],
                            in_ap=tables[b].opt()[0:bktsz, :],
                            idxs_ap=idx_all[:, s * 8:(s + k) * 8],
                            num_idxs=k * P,
                            num_idxs_reg=cnt_reg,
                            elem_size=fh,
                            single_packet=False,
                            queue_num=qq,
                        )
                        add_dep_helper(gth.ins, ld.ins, sync=False,
                                       reason="gcnt reg RAW")
                        prev_gather[0] = gth
                        gctr[0] += 1
                        oh = ohpool.tile([P, kmax, P], bf16, tag="oh")
                        nc.sync.dma_start(out=oh[:, :k, :],
                                          in_=oh_d.ap()[:, s:s + k, :])
                        ps = pspool.tile([P, P], f32, tag="ps")
                        first_visit = visit_ord[b, t] == 0
                        if first_visit:
                            # self-loop diagonal: one matmul against the
                            # core-local table rows of this tile
                            lo = t * P
                            nw = min(P, shard - lo)
                            lt = ltpool.tile([P, fh], bf16, tag="lt")
                            r0 = lo
                            while r0 < lo + nw:
                                q = r0 // qsz
                                r1 = min((q + 1) * qsz, lo + nw)
                                nc.sync.dma_start(
                                    out=lt[r0 - lo:r1 - lo, :],
                                    in_=local_slabs[q][r0 - q * qsz:
                                                       r1 - q * qsz, :])
                                r0 = r1
                            if layer1:
                                nc.tensor.matmul(
                                    out=ps[:], lhsT=lt[:],
                                    rhs=diag_sb[:, t, :], start=True,
                                    stop=False)
                            else:
                                nc.tensor.matmul(
                                    out=ps[:], lhsT=diag_sb[:, t, :],
                                    rhs=lt[:], start=True, stop=False)
                        for j in range(k):
                            st = (j == 0) and not first_visit
                            if layer1:
                                nc.tensor.matmul(
                                    out=ps[:], lhsT=gb[:, j, :],
                                    rhs=oh[:, j, :], start=st,
                                    stop=(j == k - 1))
                            else:
                                nc.tensor.matmul(
                                    out=ps[:], lhsT=oh[:, j, :],
                                    rhs=gb[:, j, :], start=st,
                                    stop=(j == k - 1))
                        asl = accs[:, t * P:(t + 1) * P]
                        if visit_ord[b, t] == 0:
                            nc.vector.tensor_copy(out=asl, in_=ps[:])
                        else:
                            nc.vector.tensor_tensor(
                                out=asl, in0=asl, in1=ps[:],
                                op=mybir.AluOpType.add)
                        if visit_ord[b, t] == nvisit[t] - 1:
                            epilogue(t, asl)

            def phase_b():
                with (
                    tc.tile_pool(name="acc1p", bufs=1) as acc1p,
                    tc.tile_pool(name="ep1", bufs=1) as ep1,
                    tc.tile_pool(name="ep1ps", bufs=2, space="PSUM") as ep1ps,
                ):
                    gsb = []
                    for i in range(3):
                        g = ep1.tile([P, P], bf16, tag=f"gsb{i}",
                                     name=f"gsb{i}")
                        nc.vector.memset(g[:], 0.0)
                        gsb.append(g)
                    zpool = [ep1.tile([P, P], bf16, tag=f"z{i}", name=f"zb{i}")
                             for i in range(3)]

                    def epi1(t, asl):
                        lo = t * P
                        nw = min(P, shard - lo)
                        z = zpool[t % 3]
                        nc.scalar.activation(
                            out=z[:], in_=asl,
                            func=mybir.ActivationFunctionType.Relu,
                            bias=b1_sb[:, 0:1])
                        ps2 = ep1ps.tile([P, fo], f32, tag="ps2")
                        nc.tensor.matmul(out=ps2[:], lhsT=z[:], rhs=w2_sb[:],
                                         start=True, stop=True)
                        g = gsb[t % 3]
                        nc.vector.tensor_copy(out=g[:, :fo], in_=ps2[:])
                        store_by_quarter(giq, g, lo, nw)

                    agg_pass([cur[f"h1f{q}"] for q in range(NQUART)], h1q,
                             True, acc1p, epi1)

            def phase_d():
                with (
                    tc.tile_pool(name="acc2p", bufs=1) as acc2p,
                    tc.tile_pool(name="ep2", bufs=3) as ep2,
                ):
                    def epi2(t, asl):
                        lo = t * P
                        nw = min(P, shard - lo)
                        o = ep2.tile([P, fo], f32, tag="o")
                        nc.vector.tensor_tensor(out=o[:], in0=asl[:, :fo],
                                                in1=b2_sb[:],
                                                op=mybir.AluOpType.add)
                        nc.sync.dma_start(out=outp.ap()[lo:lo + nw, :],
                                          in_=o[:nw, :])

                    agg_pass([cur[f"gf{q}"] for q in range(NQUART)], giq,
                             False, acc2p, epi2)

            phase_a()
            load_agg_consts()
            for q in range(NQUART):
                ag(h1q[q], fh, f"h1f{q}")
            phase_b()
            for q in range(NQUART):
                ag(giq[q], P, f"gf{q}")
            phase_d()

    nc.compile()
    return nc


# ---------------------------------------------------------------- entry point
def kernel(x, edge_index, W1, b1, W2, b2, _want_results=False, _trace=False):
    import ml_dtypes
    import concourse.bass_utils as bass_utils

    x = np.ascontiguousarray(np.asarray(x, dtype=np.float32))
    ei = np.asarray(edge_index).astype(np.int64)
    W1 = np.asarray(W1, dtype=np.float32)
    b1 = np.asarray(b1, dtype=np.float32)
    W2 = np.asarray(W2, dtype=np.float32)
    b2 = np.asarray(b2, dtype=np.float32)
    n, fin = x.shape
    fh = W1.shape[1]
    fo = W2.shape[1]

    key = ("v8", n, fin, fh, fo, int(ei[0, :8].sum()), int(ei[1, :8].sum()),
           ei.shape[1])
    if key in _CACHE:
        nc, pp, in_static = _CACHE[key]
    else:
        pp = _prep(ei, n)
        nc = _build(pp, fin, fh, fo)
        in_static = []
        for c in range(NCORES):
            in_static.append({
                "idxt": _pack_idx(pp["idx_pad"][c]),
                "oht": _build_onehots(pp, c),
                "gcnt": np.ascontiguousarray(
                    np.tile(pp["gcnt"][c:c + 1], (P, 1))),
                "diag": _build_diag(pp, c),
            })
        _CACHE[key] = (nc, pp, in_static)

    shard = pp["shard"]
    b1c = b1.reshape(fh, 1)
    b2r = np.tile(b2[None, :], (P, 1))
    W1b = W1.astype(ml_dtypes.bfloat16)
    W2b = W2.astype(ml_dtypes.bfloat16)

    in_maps = []
    for c in range(NCORES):
        xT = np.ascontiguousarray(
            x[c * shard:(c + 1) * shard, :].T.astype(ml_dtypes.bfloat16))
        m = {"xT": xT, "W1b": W1b, "W2b": W2b, "b1c": b1c, "b2r": b2r}
        m.update(in_static[c])
        in_maps.append(m)

    res = bass_utils.run_bass_kernel_spmd(
        nc, in_maps, core_ids=list(range(NCORES)), trace=_trace)
    out = np.concatenate([res.results[c]["outp"] for c in range(NCORES)],
                         axis=0)
    if _want_results:
        return out, res
    return out


# revision 30
# speedup vs baseline: 1.0398x; 1.0167x over previous
"""GCN 2-layer encoder on 8 Trainium2 NeuronCores (Bass/Tile).

Strategy (graph/data parallel, per sharding hint):
 - Nodes sharded by contiguous range across 8 cores (dst side).
 - h1 = x @ W1 (bf16) on each core's shard; the shard is split into 4
   quarters and each quarter is AllGathered separately, so aggregation for
   source-bucket q (the concat of every rank's q-th quarter, 25k rows,
   int16-indexable) can start as soon as its own AllGather lands --
   collectives overlap phase A / the previous bucket's gathers.
 - Aggregation is bucket-major: messages gathered via dma_gather
   round-robin across all 4 SWDGE queues (descriptor generation on all 8
   Q7 cores concurrently -- it is the kernel's bottleneck), scattered into
   per-tile PSUM via matmuls with host-precomputed norm-weighted bf16
   one-hots (built on DVE they stall on the DVE<->GpSimd shared SBUF
   port), accumulated across buckets in an SBUF fp32 accumulator.
 - Layer-1 epilogue fuses relu(+b1) on ScalarE, projects by W2 on PE, and
   writes zero-padded [dst, 128] bf16 rows; quarter-AllGathers of that
   table fire as epilogue quarters complete; layer 2 reuses the same
   one-hots as lhsT and adds b2.
"""
import numpy as np

NCORES = 8
P = 128
NQUART = 4

_CACHE = {}


# ---------------------------------------------------------------- preprocessing
def _prep(edge_index, n_nodes, n_cores=NCORES):
    # self-loop edges are NOT materialized: their diagonal contribution
    # dinv^2[n]*h[n] is added per tile via one matmul against diag tiles.
    # deg still counts the implicit self-loop (reference semantics).
    src = edge_index[0].astype(np.int64)
    dst = edge_index[1].astype(np.int64)
    deg = np.bincount(dst, minlength=n_nodes).astype(np.float32) + 1.0
    dinv = (1.0 / np.sqrt(deg)).astype(np.float32)
    norm = (dinv[src] * dinv[dst]).astype(np.float32)

    shard = n_nodes // n_cores
    assert shard * n_cores == n_nodes
    qsz = shard // NQUART
    assert qsz * NQUART == shard
    bktsz = qsz * n_cores  # rows per gathered quarter table
    assert bktsz <= 2 ** 15
    ntiles = (shard + P - 1) // P
    nbkt = NQUART

    core = dst // shard
    tile_id = (dst % shard) // P
    dstlocal = (dst % shard) % P
    src_r = src // shard
    src_m = src % shard
    bucket = src_m // qsz
    idx16 = (src_r * qsz + (src_m % qsz)).astype(np.int16)

    counts = np.zeros((n_cores, nbkt, ntiles), dtype=np.int64)
    np.add.at(counts, (core, bucket, tile_id), 1)
    K = (counts.max(axis=0) + P - 1) // P  # chunks per (bucket, tile)

    # (tile-block, bucket, tile) group order: bucket b's gathers still only
    # need quarter-AG b, and each quarter-block of tiles finishes all its
    # bucket visits early, so the NEXT table's quarter-AllGathers fire
    # spread across the pass instead of bunching at its end.
    tblocks = [[t for t in range(ntiles) if (t * P) // qsz == q]
               for q in range(NQUART)]
    run_off = np.zeros((nbkt, ntiles), dtype=np.int64)
    off = 0
    groups = []  # (chunk_start, n_chunks, bucket, tile)
    for tb in tblocks:
        for b in range(nbkt):
            for t in tb:
                k = int(K[b, t])
                run_off[b, t] = off * P
                if k == 0:
                    continue
                groups.append((off, k, b, t))
                off += k
    nchunks = off
    npad = nchunks * P

    nvisit = (K > 0).sum(axis=0)           # visits per tile
    visit_ord = np.cumsum(K > 0, axis=0) - 1  # visit ordinal of (b, t)

    idx_pad = np.full((n_cores, npad), -1, dtype=np.int16)
    dl_pad = np.zeros((n_cores, npad), dtype=np.int32)
    norm_pad = np.zeros((n_cores, npad), dtype=np.float32)
    for (s, k, b, t) in groups:
        idx_pad[:, s * P] = 0  # keep >=1 valid idx even for empty groups
    gcnt = np.zeros((n_cores, len(groups)), dtype=np.int32)
    gmax = np.zeros(len(groups), dtype=np.int64)
    for gi_, (s, k, b, t) in enumerate(groups):
        gcnt[:, gi_] = np.maximum(counts[:, b, t], 1)
        gmax[gi_] = counts[:, b, t].max()

    order = np.lexsort((tile_id, bucket, core))
    bucket_s, tile_sorted = bucket[order], tile_id[order]
    idx_s, dl_s, norm_s = idx16[order], dstlocal[order], norm[order]
    core_s = core[order]

    for c in range(n_cores):
        m = core_s == c
        bs, ts = bucket_s[m], tile_sorted[m]
        key = bs * ntiles + ts
        sort_idx = np.argsort(key, kind="stable")
        kk = key[sort_idx]
        boundary = np.r_[True, kk[1:] != kk[:-1]] if len(kk) else np.zeros(0, bool)
        grp_start = np.flatnonzero(boundary)
        within = np.arange(len(kk)) - np.repeat(
            grp_start, np.diff(np.r_[grp_start, len(kk)])
        )
        ranks = np.empty_like(key)
        ranks[sort_idx] = within
        slot = run_off[bs, ts] + ranks
        idx_pad[c, slot] = idx_s[m]
        dl_pad[c, slot] = dl_s[m]
        norm_pad[c, slot] = norm_s[m]

    assert nvisit.min() >= 1, "tile with no incoming edges"
    return dict(
        shard=shard, qsz=qsz, bktsz=bktsz, ntiles=ntiles, nbkt=nbkt, K=K,
        groups=groups, idx_pad=idx_pad, dl_pad=dl_pad, norm_pad=norm_pad,
        nchunks=nchunks, gcnt=gcnt, nvisit=nvisit, visit_ord=visit_ord,
        dinv=dinv, gmax=gmax,
    )


def _build_diag(pp, c):
    """diag[p, t, d] = dinv^2 of node (c*shard + t*128 + p) if p == d else 0;
    the per-tile rhs/lhsT that adds the self-loop (diagonal) contribution."""
    import ml_dtypes
    shard, ntiles = pp["shard"], pp["ntiles"]
    d2 = (pp["dinv"][c * shard:(c + 1) * shard] ** 2).astype(np.float32)
    d2 = np.pad(d2, (0, ntiles * P - shard))
    diag = np.zeros((P, ntiles, P), dtype=ml_dtypes.bfloat16)
    ti = np.repeat(np.arange(ntiles), P)
    pi = np.tile(np.arange(P), ntiles)
    diag[pi, ti, pi] = d2  # node i = ti[i]*128 + pi[i]
    return np.ascontiguousarray(diag)


def _pack_idx(idx_pad_c):
    """[npad] int16 -> [128, npad//16] wrapped in 16 partitions, replicated x8
    so every SWDGE queue's Q7 core pair finds them in its partitions."""
    npad = idx_pad_c.shape[0]
    t = idx_pad_c.reshape(npad // 16, 16).T
    return np.ascontiguousarray(np.tile(t, (8, 1)))


def _build_onehots(pp, c):
    """bf16 one-hots: oh[p, chunk, d] = norm of edge (chunk*128+p) if its
    dstlocal == d else 0."""
    import ml_dtypes
    nchunks = pp["nchunks"]
    dl = pp["dl_pad"][c].reshape(nchunks, P)
    nm = pp["norm_pad"][c].reshape(nchunks, P)
    oh = np.zeros((nchunks, P, P), dtype=ml_dtypes.bfloat16)
    ci = np.repeat(np.arange(nchunks), P)
    pi = np.tile(np.arange(P), nchunks)
    oh[ci, pi, dl.ravel()] = nm.ravel().astype(ml_dtypes.bfloat16)
    return np.ascontiguousarray(oh.transpose(1, 0, 2))


# ---------------------------------------------------------------- device build
def _build(pp, fin, fh, fo, nq=4):
    import concourse.bass as bass
    import concourse.bacc as bacc
    import concourse.tile as tile
    import concourse.mybir as mybir
    from concourse.tile_rust import add_dep_helper

    f32 = mybir.dt.float32
    bf16 = mybir.dt.bfloat16
    shard, qsz, bktsz = pp["shard"], pp["qsz"], pp["bktsz"]
    ntiles, nbkt = pp["ntiles"], pp["nbkt"]
    nchunks, K, groups = pp["nchunks"], pp["K"], pp["groups"]
    nvisit, visit_ord = pp["nvisit"], pp["visit_ord"]
    kmax = int(K.max())
    kin = fin // P
    ngroups = len(groups)

    import os
    scratch = int(os.environ.get("DMA_SCRATCH", "32768"))
    nc = bacc.Bacc("TRN2", target_bir_lowering=False, debug=False,
                   num_devices=NCORES, num_swdge_queues=nq,
                   dynamic_dma_scratch_size=scratch)
    xT = nc.dram_tensor("xT", [fin, shard], bf16, kind="ExternalInput")
    W1 = nc.dram_tensor("W1b", [fin, fh], bf16, kind="ExternalInput")
    W2 = nc.dram_tensor("W2b", [fh, fo], bf16, kind="ExternalInput")
    b1c = nc.dram_tensor("b1c", [fh, 1], f32, kind="ExternalInput")
    b2r = nc.dram_tensor("b2r", [P, fo], f32, kind="ExternalInput")
    idx_d = nc.dram_tensor("idxt", [P, nchunks * 8], mybir.dt.int16,
                           kind="ExternalInput")
    oh_d = nc.dram_tensor("oht", [P, nchunks, P], bf16, kind="ExternalInput")
    gcnt_d = nc.dram_tensor("gcnt", [P, ngroups], mybir.dt.int32,
                            kind="ExternalInput")
    diag_d = nc.dram_tensor("diag", [P, ntiles, P], bf16,
                            kind="ExternalInput")
    outp = nc.dram_tensor("outp", [shard, fo], f32, kind="ExternalOutput")

    xT_v = xT.ap().rearrange("(a p) n -> p a n", p=P)
    W1_v = W1.ap().rearrange("(a p) c -> p a c", p=P)

    with tile.TileContext(nc) as tc:
        with (
            tc.tile_pool(name="const", bufs=1) as constp,
            tc.tile_pool(name="dram", bufs=1, space="DRAM") as dram,
        ):
            w1_sb = constp.tile([P, kin, fh], bf16)
            nc.sync.dma_start(out=w1_sb[:], in_=W1_v[:])
            w2_sb = constp.tile([P, fo], bf16)
            nc.sync.dma_start(out=w2_sb[:], in_=W2.ap()[:])
            b1_sb = constp.tile([P, 1], f32)
            nc.sync.dma_start(out=b1_sb[:], in_=b1c.ap()[:])
            b2_sb = constp.tile([P, fo], f32)
            nc.sync.dma_start(out=b2_sb[:], in_=b2r.ap()[:])
            idx_all = constp.tile([P, nchunks * 8], mybir.dt.int16)
            cnt_sb = constp.tile([P, ngroups], mybir.dt.int32)
            diag_sb = constp.tile([P, ntiles, P], bf16)
            cnt_dma = None
            cnt_reg = nc.gpsimd.alloc_register("gcntr")
            prev_gather = [None]

            def load_agg_consts():
                nonlocal cnt_dma
                nc.sync.dma_start(out=idx_all[:], in_=idx_d.ap()[:])
                cnt_dma = nc.sync.dma_start(out=cnt_sb[:], in_=gcnt_d.ap()[:])
                nc.sync.dma_start(out=diag_sb[:], in_=diag_d.ap()[:])

            # per-quarter shard slabs and gathered tables
            h1q = [dram.tile([qsz, fh], bf16, name=f"h1q{q}")
                   for q in range(NQUART)]
            giq = [dram.tile([qsz, P], bf16, name=f"giq{q}")
                   for q in range(NQUART)]
            cur = {}

            def store_by_quarter(slabs, sb_tile, lo, nw):
                # route rows [lo, lo+nw) of the shard into quarter slabs
                r0 = lo
                while r0 < lo + nw:
                    q = r0 // qsz
                    r1 = min((q + 1) * qsz, lo + nw)
                    nc.sync.dma_start(
                        out=slabs[q][r0 - q * qsz:r1 - q * qsz, :],
                        in_=sb_tile[r0 - lo:r1 - lo, :])
                    r0 = r1

            def phase_a():
                blk = qsz + P  # quarter block padded to cover straddling tiles
                with (
                    tc.tile_pool(name="pa_x", bufs=2) as pa_x,
                    tc.tile_pool(name="pa_sb", bufs=8) as pa_sb,
                    tc.tile_pool(name="pa_ps", bufs=4, space="PSUM") as pa_ps,
                ):
                    xq, cur_q = None, -1
                    for t in range(ntiles):
                        lo = t * P
                        nw = min(P, shard - lo)
                        qt = lo // qsz
                        if qt != cur_q:
                            cur_q = qt
                            q0 = qt * qsz
                            qw = min(blk, shard - q0)
                            xq = pa_x.tile([P, kin, blk], bf16, tag="xq")
                            nc.sync.dma_start(out=xq[:, :, :qw],
                                              in_=xT_v[:, :, q0:q0 + qw])
                        off = lo - cur_q * qsz
                        ps = pa_ps.tile([P, fh], f32, tag="ps")
                        for a in range(kin):
                            nc.tensor.matmul(out=ps[:nw, :],
                                             lhsT=xq[:, a, off:off + nw],
                                             rhs=w1_sb[:, a, :], start=(a == 0),
                                             stop=(a == kin - 1))
                        hsb = pa_sb.tile([P, fh], bf16, tag="hsb")
                        nc.vector.tensor_copy(out=hsb[:nw, :], in_=ps[:nw, :])
                        store_by_quarter(h1q, hsb, lo, nw)

            _agn = [0]

            def ag(src_t, width, key):
                dst_t = dram.tile([bktsz, width], bf16,
                                  name=f"{key}_{_agn[0]}", addr_space="Shared")
                _agn[0] += 1
                nc.gpsimd.collective_compute(
                    "AllGather", mybir.AluOpType.bypass,
                    replica_groups=[list(range(NCORES))],
                    ins=[src_t.opt()], outs=[dst_t.opt()],
                )
                cur[key] = dst_t

            def agg_pass(tables, local_slabs, layer1, acc_pool, epilogue):
                accs = acc_pool.tile([P, ntiles * P], f32,
                                     name=f"acc{int(layer1)}")
                with (
                    tc.tile_pool(name=f"gb{int(layer1)}", bufs=1) as gpool,
                    tc.tile_pool(name=f"oh{int(layer1)}", bufs=6) as ohpool,
                    tc.tile_pool(name=f"lt{int(layer1)}", bufs=3) as ltpool,
                    tc.tile_pool(name=f"ps{int(layer1)}", bufs=3,
                                 space="PSUM") as pspool,
                ):
                    gbufs = []
                    for bi_ in range(16):
                        gz = gpool.tile([P, kmax, fh], bf16, tag=f"gb{bi_}",
                                        name=f"gbuf{int(layer1)}_{bi_}")
                        nc.vector.memset(gz[:], 0.0)
                        gbufs.append(gz)
                    gctr = [0]
                    for gi_, (s, k, b, t) in enumerate(groups):
                        gb = gbufs[gctr[0] % 16]
                        qq = gctr[0] % 4  # keep queue<->sem-lane congruence
                        ld = nc.gpsimd.load(cnt_reg, cnt_sb[0:1, gi_:gi_ + 1])
                        if prev_gather[0] is not None:
                            add_dep_helper(ld.ins, prev_gather[0].ins,
                                           sync=False, reason="gcnt reg WAR")
                        else:
                            add_dep_helper(ld.ins, cnt_dma.ins, sync=True,
                                           reason="gcnt sbuf RAW")
                        gth = nc.gpsimd.dma_gather(
                            out_ap=gb[:, :k, :],
                            in_ap=tables[b].opt()[0:bktsz, :],
                            idxs_ap=idx_all[:, s * 8:(s + k) * 8],
                            num_idxs=k * P,
                            num_idxs_reg=cnt_reg,
                            elem_size=fh,
                            single_packet=False,
                            queue_num=qq,
                        )
                        add_dep_helper(gth.ins, ld.ins, sync=False,
                                       reason="gcnt reg RAW")
                        prev_gather[0] = gth
                        gctr[0] += 1
                        oh = ohpool.tile([P, kmax, P], bf16, tag="oh")
                        nc.sync.dma_start(out=oh[:, :k, :],
                                          in_=oh_d.ap()[:, s:s + k, :])
                        ps = pspool.tile([P, P], f32, tag="ps")
                        first_visit = visit_ord[b, t] == 0
                        if first_visit:
                            # self-loop diagonal: one matmul against the
                            # core-local table rows of this tile
                            lo = t * P
                            nw = min(P, shard - lo)
                            lt = ltpool.tile([P, fh], bf16, tag="lt")
                            r0 = lo
                            while r0 < lo + nw:
                                q = r0 // qsz
                                r1 = min((q + 1) * qsz, lo + nw)
                                nc.sync.dma_start(
                                    out=lt[r0 - lo:r1 - lo, :],
                                    in_=local_slabs[q][r0 - q * qsz:
                                                       r1 - q * qsz, :])
                                r0 = r1
                            if layer1:
                                nc.tensor.matmul(
                                    out=ps[:], lhsT=lt[:],
                                    rhs=diag_sb[:, t, :], start=True,
                                    stop=False)
                            else:
                                nc.tensor.matmul(
                                    out=ps[:], lhsT=diag_sb[:, t, :],
                                    rhs=lt[:], start=True, stop=False)
                        for j in range(k):
                            st = (j == 0) and not first_visit
                            if layer1:
                                nc.tensor.matmul(
                                    out=ps[:], lhsT=gb[:, j, :],
                                    rhs=oh[:, j, :], start=st,
                                    stop=(j == k - 1))
                            else:
                                nc.tensor.matmul(
                                    out=ps[:], lhsT=oh[:, j, :],
                                    rhs=gb[:, j, :], start=st,
                                    stop=(j == k - 1))
                        asl = accs[:, t * P:(t + 1) * P]
                        if visit_ord[b, t] == 0:
                            nc.vector.tensor_copy(out=asl, in_=ps[:])
                        else:
                            nc.vector.tensor_tensor(
                                out=asl, in0=asl, in1=ps[:],
                                op=mybir.AluOpType.add)
                        if visit_ord[b, t] == nvisit[t] - 1:
                            epilogue(t, asl)

            def phase_b():
                with (
                    tc.tile_pool(name="acc1p", bufs=1) as acc1p,
                    tc.tile_pool(name="ep1", bufs=1) as ep1,
                    tc.tile_pool(name="ep1ps", bufs=2, space="PSUM") as ep1ps,
                ):
                    gsb = []
                    for i in range(3):
                        g = ep1.tile([P, P], bf16, tag=f"gsb{i}",
                                     name=f"gsb{i}")
                        nc.vector.memset(g[:], 0.0)
                        gsb.append(g)
                    zpool = [ep1.tile([P, P], bf16, tag=f"z{i}", name=f"zb{i}")
                             for i in range(3)]

                    def epi1(t, asl):
                        lo = t * P
                        nw = min(P, shard - lo)
                        z = zpool[t % 3]
                        nc.scalar.activation(
                            out=z[:], in_=asl,
                            func=mybir.ActivationFunctionType.Relu,
                            bias=b1_sb[:, 0:1])
                        ps2 = ep1ps.tile([P, fo], f32, tag="ps2")
                        nc.tensor.matmul(out=ps2[:], lhsT=z[:], rhs=w2_sb[:],
                                         start=True, stop=True)
                        g = gsb[t % 3]
                        nc.vector.tensor_copy(out=g[:, :fo], in_=ps2[:])
                        store_by_quarter(giq, g, lo, nw)

                    agg_pass([cur[f"h1f{q}"] for q in range(NQUART)], h1q,
                             True, acc1p, epi1)

            def phase_d():
                with (
                    tc.tile_pool(name="acc2p", bufs=1) as acc2p,
                    tc.tile_pool(name="ep2", bufs=3) as ep2,
                ):
                    def epi2(t, asl):
                        lo = t * P
                        nw = min(P, shard - lo)
                        o = ep2.tile([P, fo], f32, tag="o")
                        nc.vector.tensor_tensor(out=o[:], in0=asl[:, :fo],
                                                in1=b2_sb[:],
                                                op=mybir.AluOpType.add)
                        nc.sync.dma_start(out=outp.ap()[lo:lo + nw, :],
                                          in_=o[:nw, :])

                    agg_pass([cur[f"gf{q}"] for q in range(NQUART)], giq,
                             False, acc2p, epi2)

            phase_a()
            load_agg_consts()
            for q in range(NQUART):
                ag(h1q[q], fh, f"h1f{q}")
            phase_b()
            for q in range(NQUART):
                ag(giq[q], P, f"gf{q}")
            phase_d()

    nc.compile()
    return nc


# ---------------------------------------------------------------- entry point
def kernel(x, edge_index, W1, b1, W2, b2, _want_results=False, _trace=False):
    import ml_dtypes
    import concourse.bass_utils as bass_utils

    x = np.ascontiguousarray(np.asarray(x, dtype=np.float32))
    ei = np.asarray(edge_index).astype(np.int64)
    W1 = np.asarray(W1, dtype=np.float32)
    b1 = np.asarray(b1, dtype=np.float32)
    W2 = np.asarray(W2, dtype=np.float32)
    b2 = np.asarray(b2, dtype=np.float32)
    n, fin = x.shape
    fh = W1.shape[1]
    fo = W2.shape[1]

    key = ("v9", n, fin, fh, fo, int(ei[0, :8].sum()), int(ei[1, :8].sum()),
           ei.shape[1])
    if key in _CACHE:
        nc, pp, in_static = _CACHE[key]
    else:
        pp = _prep(ei, n)
        nc = _build(pp, fin, fh, fo)
        in_static = []
        for c in range(NCORES):
            in_static.append({
                "idxt": _pack_idx(pp["idx_pad"][c]),
                "oht": _build_onehots(pp, c),
                "gcnt": np.ascontiguousarray(
                    np.tile(pp["gcnt"][c:c + 1], (P, 1))),
                "diag": _build_diag(pp, c),
            })
        _CACHE[key] = (nc, pp, in_static)

    shard = pp["shard"]
    b1c = b1.reshape(fh, 1)
    b2r = np.tile(b2[None, :], (P, 1))
    W1b = W1.astype(ml_dtypes.bfloat16)
    W2b = W2.astype(ml_dtypes.bfloat16)

    in_maps = []
    for c in range(NCORES):
        xT = np.ascontiguousarray(
            x[c * shard:(c + 1) * shard, :].T.astype(ml_dtypes.bfloat16))
        m = {"xT": xT, "W1b": W1b, "W2b": W2b, "b1c": b1c, "b2r": b2r}
        m.update(in_static[c])
        in_maps.append(m)

    res = bass_utils.run_bass_kernel_spmd(
        nc, in_maps, core_ids=list(range(NCORES)), trace=_trace)
    out = np.concatenate([res.results[c]["outp"] for c in range(NCORES)],
                         axis=0)
    if _want_results:
        return out, res
    return out
